# revision 19
# baseline (speedup 1.0000x reference)
"""Trainium2 Bass kernel for nn_Net_39230231281866 (dense_cnn).

Network: conv3x3(1->6) -> Taylor-sigmoid -> conv3x3(6->7) -> flatten
         -> fc(4032->128) -> sigmoid -> fc(128->10) -> log_softmax,
batch 8192, data-parallel over 8 NeuronCores (1024 samples/core).

Mapping (v2, fp8 DoubleRow):
  * conv2+fc1 folded on the host into one dense GEMM W_comb [128, 4056].
  * conv1 = banded-weight matmul per 128-output tile (36 tiles); both conv1
    and the W_comb GEMM run in fp8e4m3 with MatmulPerfMode.DoubleRow
    (2 contraction rows/cycle): the K dim is split into 2 interleaved
    k-tiles packed along the free dim ([K/2, 2, N] moving, [K/2, 2, M]
    stationary). PSUM accumulation stays fp32. End-to-end quantization
    error ~3e-4 rel (gate is 2e-2).
  * conv1 weights are scaled by -8 (= 16 * -1/2); the Taylor denominator
    custom DVE op folds the 1/16 back in: with u = (psum - 8*b1)/16,
    den16(u) = (u(u+1)+1)^2 + u + 2 = u^4+2u^3+3u^2+3u+3  (8/8 v3 stages).
  * All 18 reciprocal ops run on the ScalarE (reciprocal_and_small table),
    writing s as fp8e4 directly; W_comb is scaled by 96 = 64*1.5 (64 keeps
    fp8 weights out of denormals, 1.5 is the den16 normalization), undone
    by the scale of the tail's Exp.
  * Tail sigmoid = 1/(1+exp(-z)) via ACT Exp + DVE reciprocal_approx_fast,
    so the whole tail only needs the natural_log_exp_and_others table:
    exactly 2 ACT table loads per kernel.
  * Input windows are pre-packed on the host into per-quad (4 tiles at
    partition bases 0/32/64/96) dense [128, 1024] fp8 blocks - one DMA
    per quad, issued on the GpSimd queue (cheap dispatch).
"""

import os
import numpy as np
import ml_dtypes

_B = 8192
_NCORES = 8
_PC = _B // _NCORES
_SLICE = 512
_NSL = _PC // _SLICE

# conv1 output tiling: 26 = 3*8+2 rows, 26 = 7*3+5 cols
_OY_T = [(0, 3), (3, 3), (6, 3), (9, 3), (12, 3), (15, 3), (18, 3), (21, 3), (24, 2)]
_OX_T = [(0, 7), (7, 7), (14, 7), (21, 5)]

# class -> (K2 = padded half-window-rows). padded K = 2*K2.
_CLS_LIST = [(3, 7), (3, 5), (2, 7), (2, 5)]
_CLS_K2 = {(3, 7): 23, (3, 5): 18, (2, 7): 18, (2, 5): 18}

LAST_RESULTS = None


def _tiles():
    ts = []
    for (oy0, noy) in _OY_T:
        for (ox0, nox) in _OX_T:
            ts.append(dict(oy0=oy0, noy=noy, ox0=ox0, nox=nox,
                           ky=noy + 2, kx=nox + 2,
                           K=(noy + 2) * (nox + 2), M=noy * nox * 6,
                           cls=(noy, nox)))
    # class-major so that quads (4 consecutive tiles) share a class where
    # possible: 24x(3,7) -> quads 0-5, 8x(3,5) -> quads 6-7,
    # 3x(2,7)+1x(2,5) -> quad 8.
    order = {(3, 7): 0, (3, 5): 1, (2, 7): 2, (2, 5): 3}
    ts.sort(key=lambda t: order[t["cls"]])
    return ts


def _q8(a):
    return np.asarray(a, np.float32).astype(ml_dtypes.float8_e4m3fn)


def _host_prep(x, w1, b1, w2, b2, fw1, fb1, fw2, fb2):
    x = np.asarray(x, np.float32)
    w1 = np.asarray(w1, np.float32); b1 = np.asarray(b1, np.float32)
    w2 = np.asarray(w2, np.float32); b2 = np.asarray(b2, np.float32)
    fw1 = np.asarray(fw1, np.float32); fb1 = np.asarray(fb1, np.float32)
    fw2 = np.asarray(fw2, np.float32); fb2 = np.asarray(fb2, np.float32)

    tiles = _tiles()
    cls_idx = {c: i for i, c in enumerate(_CLS_LIST)}

    # banded conv1 weights scaled by -8 (psum = -8*conv(x)), in DoubleRow
    # interleave, replicated at the 4 partition bases:
    # w1quad[32g + r, ci*256 + j*128 + m] = w1packS[j*K2 + r, m]
    f8 = ml_dtypes.float8_e4m3fn
    w1quad = np.zeros((128, 4 * 256), np.float32)
    biaspackS = np.zeros((128, 4), np.float32)
    for cls in _CLS_LIST:
        noy, nox = cls
        kx = nox + 2
        ci = cls_idx[cls]
        K2 = _CLS_K2[cls]
        wp = np.zeros((2 * K2, 128), np.float32)
        for oy in range(noy):
            for ox in range(nox):
                for oc in range(6):
                    m = (oy * nox + ox) * 6 + oc
                    biaspackS[m, ci] = -8.0 * b1[oc]
                    for dy in range(3):
                        for dx in range(3):
                            k = (oy + dy) * kx + (ox + dx)
                            wp[k, m] = -8.0 * w1[oc, 0, dy, dx]
        for g in range(4):
            for j in range(2):
                w1quad[32 * g:32 * g + K2, ci * 256 + j * 128:
                       ci * 256 + j * 128 + 128] = wp[j * K2:(j + 1) * K2, :]

    # fold conv2 + fc1 -> W_comb [128, 6*26*26] (x96 = 64*1.5), b_comb
    fw1r = fw1.reshape(128, 7, 24, 24)
    Wc = np.zeros((128, 6, 26, 26), np.float32)
    for dy in range(3):
        for dx in range(3):
            Wc[:, :, dy:dy + 24, dx:dx + 24] += np.einsum(
                "joyx,oi->jiyx", fw1r, w2[:, :, dy, dx], optimize=True)
    b_comb = fb1 + np.einsum("joyx,o->j", fw1r, b2)
    Wc_flat = (96.0 * Wc.reshape(128, 6 * 26 * 26)).astype(np.float32)

    # W_comb columns in DoubleRow pair-interleave, fp8:
    # wcpack[p, pair*256 + j*128 + m] = Wc_flat[m, rowsel_{2*pair+j}[p]]
    wcpack = np.zeros((128, 256 * 18), np.float32)
    for t_i, t in enumerate(tiles):
        rows = []
        for oy in range(t["noy"]):
            for ox in range(t["nox"]):
                for oc in range(6):
                    rows.append((oc * 26 + t["oy0"] + oy) * 26 + t["ox0"] + ox)
        pair, j = divmod(t_i, 2)
        wcpack[:t["M"], pair * 256 + j * 128: pair * 256 + j * 128 + 128] = \
            Wc_flat[:, rows].T

    # merged f32 const blob: [biaspackS | bcombN | fb2r]
    cb32 = np.concatenate(
        [biaspackS, (-b_comb).reshape(128, 1).astype(np.float32),
         np.tile(fb2.reshape(1, 10), (128, 4)).astype(np.float32)], axis=1)
    consts = dict(
        w1quad=_q8(w1quad), wcpack=_q8(wcpack),
        cb32=np.ascontiguousarray(cb32),
        cls_idx=cls_idx,
        fw2t=np.ascontiguousarray(fw2.T).astype(np.float16),            # [128, 10]
    )

    # pre-windowed fp8 input in per-quad dense blocks:
    # blob[(sl*9 + qd)*128 + 32g + r, j*512 + n] =
    #     x_window_row(tile 4qd+g, j*K2 + r)[sample sl*512 + n]
    x_pm = _q8(x.reshape(_B, 784).T)                                    # [784, B]
    tile_wins = []
    for t in tiles:
        K2 = _CLS_K2[t["cls"]]
        rows = ((np.arange(t["ky"])[:, None] + t["oy0"]) * 28 +
                (np.arange(t["kx"])[None, :] + t["ox0"])).reshape(-1)
        w = np.zeros((2 * K2, _B), f8)
        w[:t["K"], :] = x_pm[rows, :]
        tile_wins.append(w.reshape(2, K2, _B))                          # [2, K2, B]
    return tile_wins, consts, tiles


def _register_taylor_den16s():
    import concourse.dve_ops as dve_ops
    name = "TAYLOR_DEN16S_ANT"
    if name in dve_ops._SUB_OPCODE_FOR_NAME:
        return next(o for o in dve_ops.OPS if o.name == name)
    from concourse.dve_spec import Spec, Src0, C0, C1, C2, One, lower, sq
    from concourse.dve_uop import DveOpSpec

    # u = (in0 + s0) * s1;  out = (u*(u+1)+1)^2 + u + imm2
    #   == u^4 + 2u^3 + 3u^2 + 3u + (1 + imm2)   (imm2 = 2 -> den16)
    u = (Src0 + C0) * C1
    body = sq(u * (u + One) + One) + u + C2

    def _ref(in0, in1, s0, s1, imm2):
        uu = (in0.astype(np.float32) + s0) * s1
        return (uu * (uu + 1.0) + 1.0) ** 2 + uu + imm2

    spec = Spec(body=body, reference=_ref)
    row = max(dve_ops._SUB_OPCODE_FOR_NAME.values()) + 1
    assert row < 0x20
    shas = {ver: DveOpSpec(name=name, opcode=row, uops=lower(spec, ver=ver),
                           rd1_en=False).sha(ver)
            for ver in ("v3", "v4")}
    op = dve_ops.DveOp(name, spec, subdim=False, uops_sha=shas)
    dve_ops.OPS.append(op)
    dve_ops.CUSTOM_DVE_SPECS[op.name] = op.spec
    dve_ops._SUB_OPCODE_FOR_NAME[op.name] = row
    return op


def _pin_act_tables():
    """Pin Copy -> reciprocal_and_small and Exp/Ln ->
    natural_log_exp_and_others so the kernel loads exactly 2 ACT tables."""
    import concourse.bacc as bacc
    import concourse.mybir as mybir
    if getattr(bacc, "_ant_tables_pinned", False):
        return
    orig = bacc.get_activation_tables
    AF = mybir.ActivationFunctionType

    def patched(arch):
        tabs = {k: set(v) for k, v in orig(arch).items()}
        for name, fns in tabs.items():
            if name != "natural_log_exp_and_others":
                fns.discard(AF.Exp)
                fns.discard(AF.Ln)
            if name != "reciprocal_and_small":
                fns.discard(AF.Copy)
        return tabs

    bacc.get_activation_tables = patched
    bacc._ant_tables_pinned = True


def _act_raw(nc, out, in_, func, bias=0.0, scale=1.0):
    """Emit InstActivation directly (used for Reciprocal, which the
    nc.scalar.activation wrapper refuses; measured ~1.2e-5 rel err)."""
    import concourse.mybir as mybir
    eng = nc.scalar
    inputs = [eng.lower_ap(in_)]
    for arg in (bias, scale, 0.0):
        inputs.append(mybir.ImmediateValue(dtype=mybir.dt.float32,
                                           value=float(arg)))
    return eng.add_instruction(mybir.InstActivation(
        name=nc.get_next_instruction_name(), func=func, ins=inputs,
        outs=[eng.lower_ap(out)]))


def _build_program(tiles, cls_idx):
    import concourse.bacc as bacc
    import concourse.mybir as mybir
    from concourse.tile import TileContext
    from concourse.tile_rust import add_dep_helper
    from concourse.alu_op_type import AluOpType
    from concourse.dve_ops import RECIP_APPROX_FAST_CONSTS as RC
    from concourse.dve_ops import RECIPROCAL_APPROX_FAST

    f32 = mybir.dt.float32
    f16 = mybir.dt.float16
    f8 = mybir.dt.float8e4
    AF = mybir.ActivationFunctionType
    DR = mybir.MatmulPerfMode.DoubleRow
    den_op = _register_taylor_den16s()
    _pin_act_tables()

    nc = bacc.Bacc()
    n_tiles = len(tiles)
    n_quads = n_tiles // 4
    xwin = nc.declare_dram_parameter("xwin", [128, _NSL * n_quads * 1024], f8,
                                     isOutput=False)
    w1quad_d = nc.declare_dram_parameter("w1quad", [128, 1024], f8,
                                         isOutput=False)
    wcpack_d = nc.declare_dram_parameter("wcpack", [128, 256 * 18], f8,
                                         isOutput=False)
    cb32_d = nc.declare_dram_parameter("cb32", [128, 45], f32, isOutput=False)
    fw2t_d = nc.declare_dram_parameter("fw2t", [128, 10], f16, isOutput=False)
    out_d = nc.declare_dram_parameter("out", [_PC, 10], f32, isOutput=True)

    # xwin DMA granularity: quads per DMA (consumption order)
    _XDMA = 4

    with TileContext(nc) as tc:
        with (
            tc.tile_pool(name="const", bufs=1) as cpool,
            tc.tile_pool(name="xq", bufs=5) as xpool,
            tc.tile_pool(name="q", bufs=2) as qpool,
            tc.tile_pool(name="s", bufs=2) as spool,
            tc.tile_pool(name="work", bufs=3) as wpool,
            tc.tile_pool(name="cps", bufs=2, space="PSUM") as cps,
            tc.tile_pool(name="zps", bufs=2, space="PSUM") as zps,
            tc.tile_pool(name="fps", bufs=2, space="PSUM") as fps,
        ):
            # DMA order: w1quad first (first conv1 needs it), then input
            # windows, then the consts only needed later in the pipeline.
            w1quad_sb = cpool.tile_from(w1quad_d[:], name="w1quad_sb")
            n_blk = _NSL * n_quads
            xdma = []
            for b0 in range(0, n_blk, _XDMA):
                nb = min(_XDMA, n_blk - b0)
                t = xpool.tile([128, 1024 * _XDMA], f8, tag="xq",
                               name=f"xq{b0}")
                nc.sync.dma_start(
                    out=t[:, 0:1024 * nb],
                    in_=xwin[:, b0 * 1024:(b0 + nb) * 1024])
                xdma.append(t)
            wcpack_sb = cpool.tile_from(wcpack_d[:], name="wcpack_sb")
            cb32_sb = cpool.tile_from(cb32_d[:], name="cb32_sb")
            biasp_sb = cb32_sb[:, 0:4]
            bcombN_sb = cb32_sb[:, 4:5]
            fb2r_sb = cb32_sb[:, 5:45]
            fw2t_sb = cpool.tile_from(fw2t_d[:], name="fw2t_sb")

            def quad_ap(sl, qd):
                blk = sl * n_quads + qd
                return xdma[blk // _XDMA][
                    :, (blk % _XDMA) * 1024:(blk % _XDMA) * 1024 + 1024]

            # z psum tiles (also the dummy-matmul target for the
            # single-sync-wait preamble)
            zs = [zps.tile([128, _SLICE], f32, tag="z", name=f"z{sl}")
                  for sl in range(_NSL)]

            # single-sync-wait rule: pre-observe PE-read const queues with
            # dummy 1-col matmuls; DVE/ACT-read consts with dummy touches.
            nc.tensor.matmul(zs[0][0:128, 0:1], w1quad_sb[0:45, 0:128],
                             w1quad_sb[0:45, 0:1], start=True, stop=True)
            nc.tensor.matmul(zs[0][0:128, 0:1], wcpack_sb[0:128, 0:128],
                             wcpack_sb[0:128, 0:1], start=True, stop=True)
            nc.tensor.matmul(zs[0][0:10, 0:1], fw2t_sb[0:128, 0:10],
                             fw2t_sb[0:128, 0:1], start=True, stop=True)
            dvescr = wpool.tile([128, 44], f32, tag="dvescr", name="dvescr",
                                bufs=1)
            nc.vector.tensor_copy(out=dvescr[:, 0:4], in_=biasp_sb[:])
            nc.vector.tensor_copy(out=dvescr[:, 4:44], in_=fb2r_sb[:])
            actscr = wpool.tile([128, 1], f32, tag="actscr", name="actscr",
                                bufs=1)
            nc.scalar.copy(out=actscr[:], in_=bcombN_sb[:])

            for sl in range(_NSL):
                recip_insts = []
                for qd in range(n_quads):
                    quad = quad_ap(sl, qd)
                    q = qpool.tile([128, 2048], f32, tag="q",
                                   name=f"q{sl}_{qd}")
                    s = spool.tile([128, 2048], f8, tag="s", name=f"s{sl}_{qd}")
                    for pair in range(2):
                        cp = cps.tile([128, 1024], f32, tag="cp",
                                      name=f"cp{sl}_{qd}_{pair}")
                        for j in range(2):
                            g = 2 * pair + j
                            t = tiles[4 * qd + g]
                            K2 = _CLS_K2[t["cls"]]
                            ci = cls_idx[t["cls"]]
                            rhs = quad[32 * g:32 * g + K2, :].rearrange(
                                "p (two n) -> p two n", two=2)
                            lhsT = w1quad_sb[32 * g:32 * g + K2,
                                             ci * 256:ci * 256 + 256].rearrange(
                                "p (two m) -> p two m", two=2)
                            nc.tensor.matmul(
                                cp[:, j * _SLICE:(j + 1) * _SLICE], lhsT, rhs,
                                start=True, stop=True, perf_mode=DR,
                                tile_position=(32 * g, 0))
                        ca = cls_idx[tiles[4 * qd + 2 * pair]["cls"]]
                        cb = cls_idx[tiles[4 * qd + 2 * pair + 1]["cls"]]
                        qh = q[:, pair * 1024:(pair + 1) * 1024]
                        if ca == cb:
                            nc.vector._custom_dve(
                                den_op, out=qh, in0=cp,
                                s0=biasp_sb[0:128, ca:ca + 1],
                                s1=1.0 / 16.0, imm2=2.0)
                        else:
                            nc.vector._custom_dve(
                                den_op, out=qh[:, 0:_SLICE],
                                in0=cp[:, 0:_SLICE],
                                s0=biasp_sb[0:128, ca:ca + 1],
                                s1=1.0 / 16.0, imm2=2.0)
                            nc.vector._custom_dve(
                                den_op, out=qh[:, _SLICE:1024],
                                in0=cp[:, _SLICE:1024],
                                s0=biasp_sb[0:128, cb:cb + 1],
                                s1=1.0 / 16.0, imm2=2.0)
                    ri = _act_raw(nc, s, q, AF.Reciprocal)
                    recip_insts.append(ri)
                    for pair in range(2):
                        pi = 2 * qd + pair
                        lhsT = wcpack_sb[:, pi * 256:(pi + 1) * 256].rearrange(
                            "p (two m) -> p two m", two=2)
                        rhs = s[:, pair * 1024:(pair + 1) * 1024].rearrange(
                            "p (two n) -> p two n", two=2)
                        nc.tensor.matmul(zs[sl], lhsT, rhs,
                                         start=(pi == 0), stop=(pi == 17),
                                         perf_mode=DR)

                # ---- tail for this slice: sigmoid via Exp + fast-reciprocal,
                # fc2, log_softmax. Emitted per-slice so slice 0's tail
                # overlaps slice 1's main loop (costs 2 extra ACT table
                # loads, wins the tail-0 drain). (no max-sub: |logits| < 12,
                # exp cannot overflow fp32.)
                last_recip = recip_insts[-1]
                e = wpool.tile([128, _SLICE], f32, tag="e", name=f"e{sl}")
                ei = nc.scalar.activation(e, zs[sl], AF.Exp, bias=bcombN_sb[:],
                                          scale=-1.0 / 64.0)
                add_dep_helper(ei.ins, last_recip.ins, sync=False,
                               reason="keep tail ACT after recips (table sets)")
                t1 = wpool.tile([128, _SLICE], f32, tag="t1", name=f"t1{sl}")
                t1i = nc.scalar.activation(t1, e, AF.Identity, bias=1.0,
                                           scale=1.0)
                add_dep_helper(t1i.ins, last_recip.ins, sync=False,
                               reason="keep tail ACT after recips (table sets)")
                h = wpool.tile([128, _SLICE], f16, tag="h", name=f"h{sl}")
                nc.vector._custom_dve(RECIPROCAL_APPROX_FAST, out=h, in0=t1,
                                      s0=RC["s0"], s1=RC["s1"], imm2=RC["imm2"])
                ng = _SLICE // 128
                fp = fps.tile([128, 10 * ng], f32, tag="fp", name=f"fp{sl}")
                for g in range(ng):
                    nc.tensor.matmul(fp[:, g * 10:(g + 1) * 10],
                                     h[:, g * 128:(g + 1) * 128], fw2t_sb[:],
                                     start=True, stop=True)
                lg = wpool.tile([128, 10 * ng], f32, tag="lg", name=f"lg{sl}")
                nc.vector.tensor_tensor(out=lg, in0=fp, in1=fb2r_sb[:, 0:10 * ng],
                                        op=AluOpType.add)
                e2 = wpool.tile([128, 10 * ng], f32, tag="e2", name=f"e2{sl}")
                e2i = nc.scalar.activation(e2, lg, AF.Exp)
                add_dep_helper(e2i.ins, last_recip.ins, sync=False,
                               reason="keep tail ACT after recips (table sets)")
                ssum = wpool.tile([128, ng], f32, tag="ss", name=f"ss{sl}")
                nc.vector.tensor_reduce(
                    ssum, e2.rearrange("p (g k) -> p g k", k=10),
                    axis=mybir.AxisListType.X, op=AluOpType.add)
                lns = wpool.tile([128, ng], f32, tag="ls", name=f"ls{sl}")
                li = nc.scalar.activation(lns, ssum, AF.Ln)
                add_dep_helper(li.ins, last_recip.ins, sync=False,
                               reason="keep tail ACT after recips (table sets)")
                ot = wpool.tile([128, 10 * ng], f32, tag="ot", name=f"ot{sl}")
                for g in range(ng):
                    nc.vector.tensor_scalar(
                        out=ot[:, g * 10:(g + 1) * 10],
                        in0=lg[:, g * 10:(g + 1) * 10],
                        scalar1=lns[:, g:g + 1], scalar2=None,
                        op0=AluOpType.subtract)
                orow = sl * _SLICE
                nc.sync.dma_start(
                    out=out_d[orow:orow + _SLICE, :].rearrange(
                        "(g p) k -> p g k", p=128),
                    in_=ot.rearrange("p (g k) -> p g k", k=10))
    nc.compile()
    return nc


_PROGRAM_CACHE = {}


def kernel(x, w1, b1, w2, b2, fw1, fb1, fw2, fb2):
    global LAST_RESULTS
    tile_wins, consts, tiles = _host_prep(x, w1, b1, w2, b2, fw1, fb1, fw2, fb2)

    if "nc" not in _PROGRAM_CACHE:
        _PROGRAM_CACHE["nc"] = _build_program(tiles, consts["cls_idx"])
    nc = _PROGRAM_CACHE["nc"]

    f8 = ml_dtypes.float8_e4m3fn
    n_quads = len(tiles) // 4
    shared = {k: consts[k] for k in ("w1quad", "wcpack", "cb32", "fw2t")}
    in_maps = []
    for c in range(_NCORES):
        m = dict(shared)
        blob = np.zeros((128, _NSL * n_quads * 1024), f8)
        for sl in range(_NSL):
            for qd in range(n_quads):
                col0 = (sl * n_quads + qd) * 1024
                for g in range(4):
                    t = tiles[4 * qd + g]
                    K2 = _CLS_K2[t["cls"]]
                    w = tile_wins[4 * qd + g]     # [2, K2, B]
                    c0 = c * _PC + sl * _SLICE
                    for j in range(2):
                        blob[32 * g:32 * g + K2,
                             col0 + j * _SLICE:col0 + (j + 1) * _SLICE] = \
                            w[j, :, c0:c0 + _SLICE]
        m["xwin"] = blob
        in_maps.append(m)

    from concourse.bass_utils import run_bass_kernel_spmd
    trace = bool(int(os.environ.get("BASS_KERNEL_TRACE", "0")))
    res = run_bass_kernel_spmd(nc, in_maps, core_ids=list(range(_NCORES)),
                               trace=trace)
    LAST_RESULTS = res
    return np.concatenate([r["out"] for r in res.results], axis=0)


# revision 21
# speedup vs baseline: 1.0781x; 1.0781x over previous
"""Trainium2 Bass kernel for nn_Net_39230231281866 (dense_cnn).

Network: conv3x3(1->6) -> Taylor-sigmoid -> conv3x3(6->7) -> flatten
         -> fc(4032->128) -> sigmoid -> fc(128->10) -> log_softmax,
batch 8192, data-parallel over 8 NeuronCores (1024 samples/core).

Mapping (v2, fp8 DoubleRow):
  * conv2+fc1 folded on the host into one dense GEMM W_comb [128, 4056].
  * conv1 = banded-weight matmul per 128-output tile (36 tiles); both conv1
    and the W_comb GEMM run in fp8e4m3 with MatmulPerfMode.DoubleRow
    (2 contraction rows/cycle): the K dim is split into 2 interleaved
    k-tiles packed along the free dim ([K/2, 2, N] moving, [K/2, 2, M]
    stationary). PSUM accumulation stays fp32. End-to-end quantization
    error ~3e-4 rel (gate is 2e-2).
  * conv1 weights are scaled by -8 (= 16 * -1/2); the Taylor denominator
    custom DVE op folds the 1/16 back in: with u = (psum - 8*b1)/16,
    den16(u) = (u(u+1)+1)^2 + u + 2 = u^4+2u^3+3u^2+3u+3  (8/8 v3 stages).
  * All 18 reciprocal ops run on the ScalarE (reciprocal_and_small table),
    writing s as fp8e4 directly; W_comb is scaled by 96 = 64*1.5 (64 keeps
    fp8 weights out of denormals, 1.5 is the den16 normalization), undone
    by the scale of the tail's Exp.
  * Tail sigmoid = 1/(1+exp(-z)) via ACT Exp + DVE reciprocal_approx_fast,
    so the whole tail only needs the natural_log_exp_and_others table:
    exactly 2 ACT table loads per kernel.
  * Input windows are pre-packed on the host into per-quad (4 tiles at
    partition bases 0/32/64/96) dense [128, 1024] fp8 blocks - one DMA
    per quad, issued on the GpSimd queue (cheap dispatch).
"""

import os
import numpy as np
import ml_dtypes

_B = 8192
_NCORES = 8
_PC = _B // _NCORES
_SLICE = 512
_NSL = _PC // _SLICE

# conv1 output tiling: 26 = 3*8+2 rows, 26 = 7*3+5 cols
_OY_T = [(0, 3), (3, 3), (6, 3), (9, 3), (12, 3), (15, 3), (18, 3), (21, 3), (24, 2)]
_OX_T = [(0, 7), (7, 7), (14, 7), (21, 5)]

# class -> (K2 = padded half-window-rows). padded K = 2*K2.
_CLS_LIST = [(3, 7), (3, 5), (2, 7), (2, 5)]
_CLS_K2 = {(3, 7): 23, (3, 5): 18, (2, 7): 18, (2, 5): 18}

LAST_RESULTS = None


def _tiles():
    ts = []
    for (oy0, noy) in _OY_T:
        for (ox0, nox) in _OX_T:
            ts.append(dict(oy0=oy0, noy=noy, ox0=ox0, nox=nox,
                           ky=noy + 2, kx=nox + 2,
                           K=(noy + 2) * (nox + 2), M=noy * nox * 6,
                           cls=(noy, nox)))
    # class-major so that quads (4 consecutive tiles) share a class where
    # possible: 24x(3,7) -> quads 0-5, 8x(3,5) -> quads 6-7,
    # 3x(2,7)+1x(2,5) -> quad 8.
    order = {(3, 7): 0, (3, 5): 1, (2, 7): 2, (2, 5): 3}
    ts.sort(key=lambda t: order[t["cls"]])
    return ts


def _q8(a):
    return np.asarray(a, np.float32).astype(ml_dtypes.float8_e4m3fn)


def _host_prep(x, w1, b1, w2, b2, fw1, fb1, fw2, fb2):
    x = np.asarray(x, np.float32)
    w1 = np.asarray(w1, np.float32); b1 = np.asarray(b1, np.float32)
    w2 = np.asarray(w2, np.float32); b2 = np.asarray(b2, np.float32)
    fw1 = np.asarray(fw1, np.float32); fb1 = np.asarray(fb1, np.float32)
    fw2 = np.asarray(fw2, np.float32); fb2 = np.asarray(fb2, np.float32)

    tiles = _tiles()
    cls_idx = {c: i for i, c in enumerate(_CLS_LIST)}

    # banded conv1 weights scaled by -8 (psum = -8*conv(x)), in DoubleRow
    # interleave, replicated at the 4 partition bases:
    # w1quad[32g + r, ci*256 + j*128 + m] = w1packS[j*K2 + r, m]
    f8 = ml_dtypes.float8_e4m3fn
    w1quad = np.zeros((128, 4 * 256), np.float32)
    biaspackS = np.zeros((128, 4), np.float32)
    for cls in _CLS_LIST:
        noy, nox = cls
        kx = nox + 2
        ci = cls_idx[cls]
        K2 = _CLS_K2[cls]
        wp = np.zeros((2 * K2, 128), np.float32)
        for oy in range(noy):
            for ox in range(nox):
                for oc in range(6):
                    m = (oy * nox + ox) * 6 + oc
                    biaspackS[m, ci] = -8.0 * b1[oc]
                    for dy in range(3):
                        for dx in range(3):
                            k = (oy + dy) * kx + (ox + dx)
                            wp[k, m] = -8.0 * w1[oc, 0, dy, dx]
        for g in range(4):
            for j in range(2):
                w1quad[32 * g:32 * g + K2, ci * 256 + j * 128:
                       ci * 256 + j * 128 + 128] = wp[j * K2:(j + 1) * K2, :]

    # fold conv2 + fc1 -> W_comb [128, 6*26*26] (x96 = 64*1.5), b_comb
    fw1r = fw1.reshape(128, 7, 24, 24)
    Wc = np.zeros((128, 6, 26, 26), np.float32)
    for dy in range(3):
        for dx in range(3):
            Wc[:, :, dy:dy + 24, dx:dx + 24] += np.einsum(
                "joyx,oi->jiyx", fw1r, w2[:, :, dy, dx], optimize=True)
    b_comb = fb1 + np.einsum("joyx,o->j", fw1r, b2)
    Wc_flat = (96.0 * Wc.reshape(128, 6 * 26 * 26)).astype(np.float32)

    # W_comb columns in DoubleRow pair-interleave, fp8:
    # wcpack[p, pair*256 + j*128 + m] = Wc_flat[m, rowsel_{2*pair+j}[p]]
    wcpack = np.zeros((128, 256 * 18), np.float32)
    for t_i, t in enumerate(tiles):
        rows = []
        for oy in range(t["noy"]):
            for ox in range(t["nox"]):
                for oc in range(6):
                    rows.append((oc * 26 + t["oy0"] + oy) * 26 + t["ox0"] + ox)
        pair, j = divmod(t_i, 2)
        wcpack[:t["M"], pair * 256 + j * 128: pair * 256 + j * 128 + 128] = \
            Wc_flat[:, rows].T

    # merged f32 const blob: [biaspackS | bcombN | fb2r]
    cb32 = np.concatenate(
        [biaspackS, (-b_comb).reshape(128, 1).astype(np.float32),
         np.tile(fb2.reshape(1, 10), (128, 4)).astype(np.float32)], axis=1)
    consts = dict(
        w1quad=_q8(w1quad), wcpack=_q8(wcpack),
        cb32=np.ascontiguousarray(cb32),
        cls_idx=cls_idx,
        fw2t=np.ascontiguousarray(fw2.T).astype(np.float16),            # [128, 10]
    )

    # pre-windowed fp8 input in per-quad dense blocks:
    # blob[(sl*9 + qd)*128 + 32g + r, j*512 + n] =
    #     x_window_row(tile 4qd+g, j*K2 + r)[sample sl*512 + n]
    x_pm = _q8(x.reshape(_B, 784).T)                                    # [784, B]
    tile_wins = []
    for t in tiles:
        K2 = _CLS_K2[t["cls"]]
        rows = ((np.arange(t["ky"])[:, None] + t["oy0"]) * 28 +
                (np.arange(t["kx"])[None, :] + t["ox0"])).reshape(-1)
        w = np.zeros((2 * K2, _B), f8)
        w[:t["K"], :] = x_pm[rows, :]
        tile_wins.append(w.reshape(2, K2, _B))                          # [2, K2, B]
    return tile_wins, consts, tiles


def _register_taylor_den16s():
    import concourse.dve_ops as dve_ops
    name = "TAYLOR_DEN16S_ANT"
    if name in dve_ops._SUB_OPCODE_FOR_NAME:
        return next(o for o in dve_ops.OPS if o.name == name)
    from concourse.dve_spec import Spec, Src0, C0, C1, C2, One, lower, sq
    from concourse.dve_uop import DveOpSpec

    # u = (in0 + s0) * s1;  out = (u*(u+1)+1)^2 + u + imm2
    #   == u^4 + 2u^3 + 3u^2 + 3u + (1 + imm2)   (imm2 = 2 -> den16)
    u = (Src0 + C0) * C1
    body = sq(u * (u + One) + One) + u + C2

    def _ref(in0, in1, s0, s1, imm2):
        uu = (in0.astype(np.float32) + s0) * s1
        return (uu * (uu + 1.0) + 1.0) ** 2 + uu + imm2

    spec = Spec(body=body, reference=_ref)
    row = max(dve_ops._SUB_OPCODE_FOR_NAME.values()) + 1
    assert row < 0x20
    shas = {ver: DveOpSpec(name=name, opcode=row, uops=lower(spec, ver=ver),
                           rd1_en=False).sha(ver)
            for ver in ("v3", "v4")}
    op = dve_ops.DveOp(name, spec, subdim=False, uops_sha=shas)
    dve_ops.OPS.append(op)
    dve_ops.CUSTOM_DVE_SPECS[op.name] = op.spec
    dve_ops._SUB_OPCODE_FOR_NAME[op.name] = row
    return op


def _pin_act_tables():
    """Pin Copy -> reciprocal_and_small and Exp/Ln ->
    natural_log_exp_and_others so the kernel loads exactly 2 ACT tables."""
    import concourse.bacc as bacc
    import concourse.mybir as mybir
    if getattr(bacc, "_ant_tables_pinned", False):
        return
    orig = bacc.get_activation_tables
    AF = mybir.ActivationFunctionType

    def patched(arch):
        tabs = {k: set(v) for k, v in orig(arch).items()}
        for name, fns in tabs.items():
            if name != "natural_log_exp_and_others":
                fns.discard(AF.Exp)
                fns.discard(AF.Ln)
            if name != "reciprocal_and_small":
                fns.discard(AF.Copy)
        return tabs

    bacc.get_activation_tables = patched
    bacc._ant_tables_pinned = True


def _act_raw(nc, out, in_, func, bias=0.0, scale=1.0):
    """Emit InstActivation directly (used for Reciprocal, which the
    nc.scalar.activation wrapper refuses; measured ~1.2e-5 rel err)."""
    import concourse.mybir as mybir
    eng = nc.scalar
    inputs = [eng.lower_ap(in_)]
    for arg in (bias, scale, 0.0):
        inputs.append(mybir.ImmediateValue(dtype=mybir.dt.float32,
                                           value=float(arg)))
    return eng.add_instruction(mybir.InstActivation(
        name=nc.get_next_instruction_name(), func=func, ins=inputs,
        outs=[eng.lower_ap(out)]))


def _build_program(tiles, cls_idx):
    import concourse.bacc as bacc
    import concourse.mybir as mybir
    from concourse.tile import TileContext
    from concourse.tile_rust import add_dep_helper
    from concourse.alu_op_type import AluOpType
    from concourse.dve_ops import RECIP_APPROX_FAST_CONSTS as RC
    from concourse.dve_ops import RECIPROCAL_APPROX_FAST

    f32 = mybir.dt.float32
    f16 = mybir.dt.float16
    f8 = mybir.dt.float8e4
    AF = mybir.ActivationFunctionType
    DR = mybir.MatmulPerfMode.DoubleRow
    den_op = _register_taylor_den16s()
    _pin_act_tables()

    nc = bacc.Bacc()
    n_tiles = len(tiles)
    n_quads = n_tiles // 4
    xwin = nc.declare_dram_parameter("xwin", [128, _NSL * n_quads * 1024], f8,
                                     isOutput=False)
    w1quad_d = nc.declare_dram_parameter("w1quad", [128, 1024], f8,
                                         isOutput=False)
    wcpack_d = nc.declare_dram_parameter("wcpack", [128, 256 * 18], f8,
                                         isOutput=False)
    cb32_d = nc.declare_dram_parameter("cb32", [128, 45], f32, isOutput=False)
    fw2t_d = nc.declare_dram_parameter("fw2t", [128, 10], f16, isOutput=False)
    out_d = nc.declare_dram_parameter("out", [_PC, 10], f32, isOutput=True)

    # xwin DMA granularity: quads per DMA (consumption order)
    _XDMA = 4

    with TileContext(nc) as tc:
        with (
            tc.tile_pool(name="const", bufs=1) as cpool,
            tc.tile_pool(name="xq", bufs=5) as xpool,
            tc.tile_pool(name="q", bufs=2) as qpool,
            tc.tile_pool(name="s", bufs=2) as spool,
            tc.tile_pool(name="work", bufs=3) as wpool,
            tc.tile_pool(name="cps", bufs=2, space="PSUM") as cps,
            tc.tile_pool(name="zps", bufs=2, space="PSUM") as zps,
            tc.tile_pool(name="fps", bufs=2, space="PSUM") as fps,
        ):
            # DMA order: w1quad first (first conv1 needs it), then input
            # windows, then the consts only needed later in the pipeline.
            w1quad_sb = cpool.tile_from(w1quad_d[:], name="w1quad_sb")
            n_blk = _NSL * n_quads
            xdma = []
            for b0 in range(0, n_blk, _XDMA):
                nb = min(_XDMA, n_blk - b0)
                t = xpool.tile([128, 1024 * _XDMA], f8, tag="xq",
                               name=f"xq{b0}")
                nc.sync.dma_start(
                    out=t[:, 0:1024 * nb],
                    in_=xwin[:, b0 * 1024:(b0 + nb) * 1024])
                xdma.append(t)
            wcpack_sb = cpool.tile_from(wcpack_d[:], name="wcpack_sb")
            cb32_sb = cpool.tile_from(cb32_d[:], name="cb32_sb")
            biasp_sb = cb32_sb[:, 0:4]
            bcombN_sb = cb32_sb[:, 4:5]
            fb2r_sb = cb32_sb[:, 5:45]
            fw2t_sb = cpool.tile_from(fw2t_d[:], name="fw2t_sb")

            def quad_ap(sl, qd):
                blk = sl * n_quads + qd
                return xdma[blk // _XDMA][
                    :, (blk % _XDMA) * 1024:(blk % _XDMA) * 1024 + 1024]

            # z psum tiles (also the dummy-matmul target for the
            # single-sync-wait preamble)
            zs = [zps.tile([128, _SLICE], f32, tag="z", name=f"z{sl}")
                  for sl in range(_NSL)]

            # single-sync-wait rule: pre-observe PE-read const queues with
            # dummy 1-col matmuls; DVE/ACT-read consts with dummy touches.
            nc.tensor.matmul(zs[0][0:128, 0:1], w1quad_sb[0:45, 0:128],
                             w1quad_sb[0:45, 0:1], start=True, stop=True)
            nc.tensor.matmul(zs[0][0:128, 0:1], wcpack_sb[0:128, 0:128],
                             wcpack_sb[0:128, 0:1], start=True, stop=True)
            nc.tensor.matmul(zs[0][0:10, 0:1], fw2t_sb[0:128, 0:10],
                             fw2t_sb[0:128, 0:1], start=True, stop=True)
            dvescr = wpool.tile([128, 44], f32, tag="dvescr", name="dvescr",
                                bufs=1)
            nc.vector.tensor_copy(out=dvescr[:, 0:4], in_=biasp_sb[:])
            nc.vector.tensor_copy(out=dvescr[:, 4:44], in_=fb2r_sb[:])
            actscr = wpool.tile([128, 1], f32, tag="actscr", name="actscr",
                                bufs=1)
            nc.scalar.copy(out=actscr[:], in_=bcombN_sb[:])

            recip_insts = []
            for sl in range(_NSL):
                for qd in range(n_quads):
                    quad = quad_ap(sl, qd)
                    q = qpool.tile([128, 2048], f32, tag="q",
                                   name=f"q{sl}_{qd}")
                    s = spool.tile([128, 2048], f8, tag="s", name=f"s{sl}_{qd}")
                    for pair in range(2):
                        cp = cps.tile([128, 1024], f32, tag="cp",
                                      name=f"cp{sl}_{qd}_{pair}")
                        for j in range(2):
                            g = 2 * pair + j
                            t = tiles[4 * qd + g]
                            K2 = _CLS_K2[t["cls"]]
                            ci = cls_idx[t["cls"]]
                            rhs = quad[32 * g:32 * g + K2, :].rearrange(
                                "p (two n) -> p two n", two=2)
                            lhsT = w1quad_sb[32 * g:32 * g + K2,
                                             ci * 256:ci * 256 + 256].rearrange(
                                "p (two m) -> p two m", two=2)
                            nc.tensor.matmul(
                                cp[:, j * _SLICE:(j + 1) * _SLICE], lhsT, rhs,
                                start=True, stop=True, perf_mode=DR,
                                tile_position=(32 * g, 0))
                        ca = cls_idx[tiles[4 * qd + 2 * pair]["cls"]]
                        cb = cls_idx[tiles[4 * qd + 2 * pair + 1]["cls"]]
                        qh = q[:, pair * 1024:(pair + 1) * 1024]
                        if ca == cb:
                            nc.vector._custom_dve(
                                den_op, out=qh, in0=cp,
                                s0=biasp_sb[0:128, ca:ca + 1],
                                s1=1.0 / 16.0, imm2=2.0)
                        else:
                            nc.vector._custom_dve(
                                den_op, out=qh[:, 0:_SLICE],
                                in0=cp[:, 0:_SLICE],
                                s0=biasp_sb[0:128, ca:ca + 1],
                                s1=1.0 / 16.0, imm2=2.0)
                            nc.vector._custom_dve(
                                den_op, out=qh[:, _SLICE:1024],
                                in0=cp[:, _SLICE:1024],
                                s0=biasp_sb[0:128, cb:cb + 1],
                                s1=1.0 / 16.0, imm2=2.0)
                    ri = _act_raw(nc, s, q, AF.Reciprocal)
                    recip_insts.append(ri)
                    for pair in range(2):
                        pi = 2 * qd + pair
                        lhsT = wcpack_sb[:, pi * 256:(pi + 1) * 256].rearrange(
                            "p (two m) -> p two m", two=2)
                        rhs = s[:, pair * 1024:(pair + 1) * 1024].rearrange(
                            "p (two n) -> p two n", two=2)
                        nc.tensor.matmul(zs[sl], lhsT, rhs,
                                         start=(pi == 0), stop=(pi == 17),
                                         perf_mode=DR)

            # ---- tail: sigmoid via Exp + fast-reciprocal, fc2, log_softmax.
            # (no max-sub: |logits| < 12, exp cannot overflow fp32.)
            # NOTE: must stay after ALL recips - interleaving tail ACT ops
            # with recips thrashes the ACT table sets (measured 8 loads).
            last_recip = recip_insts[-1]
            for sl in range(_NSL):
                e = wpool.tile([128, _SLICE], f32, tag="e", name=f"e{sl}")
                ei = nc.scalar.activation(e, zs[sl], AF.Exp, bias=bcombN_sb[:],
                                          scale=-1.0 / 64.0)
                add_dep_helper(ei.ins, last_recip.ins, sync=False,
                               reason="keep tail ACT after recips (table sets)")
                t1 = wpool.tile([128, _SLICE], f32, tag="t1", name=f"t1{sl}")
                t1i = nc.scalar.activation(t1, e, AF.Identity, bias=1.0,
                                           scale=1.0)
                add_dep_helper(t1i.ins, last_recip.ins, sync=False,
                               reason="keep tail ACT after recips (table sets)")
                h = wpool.tile([128, _SLICE], f16, tag="h", name=f"h{sl}")
                nc.vector._custom_dve(RECIPROCAL_APPROX_FAST, out=h, in0=t1,
                                      s0=RC["s0"], s1=RC["s1"], imm2=RC["imm2"])
                ng = _SLICE // 128
                fp = fps.tile([128, 10 * ng], f32, tag="fp", name=f"fp{sl}")
                for g in range(ng):
                    nc.tensor.matmul(fp[:, g * 10:(g + 1) * 10],
                                     h[:, g * 128:(g + 1) * 128], fw2t_sb[:],
                                     start=True, stop=True)
                lg = wpool.tile([128, 10 * ng], f32, tag="lg", name=f"lg{sl}")
                nc.vector.tensor_tensor(out=lg, in0=fp, in1=fb2r_sb[:, 0:10 * ng],
                                        op=AluOpType.add)
                e2 = wpool.tile([128, 10 * ng], f32, tag="e2", name=f"e2{sl}")
                e2i = nc.scalar.activation(e2, lg, AF.Exp)
                add_dep_helper(e2i.ins, last_recip.ins, sync=False,
                               reason="keep tail ACT after recips (table sets)")
                ssum = wpool.tile([128, ng], f32, tag="ss", name=f"ss{sl}")
                nc.vector.tensor_reduce(
                    ssum, e2.rearrange("p (g k) -> p g k", k=10),
                    axis=mybir.AxisListType.X, op=AluOpType.add)
                lns = wpool.tile([128, ng], f32, tag="ls", name=f"ls{sl}")
                li = nc.scalar.activation(lns, ssum, AF.Ln)
                add_dep_helper(li.ins, last_recip.ins, sync=False,
                               reason="keep tail ACT after recips (table sets)")
                ot = wpool.tile([128, 10 * ng], f32, tag="ot", name=f"ot{sl}")
                for g in range(ng):
                    nc.vector.tensor_scalar(
                        out=ot[:, g * 10:(g + 1) * 10],
                        in0=lg[:, g * 10:(g + 1) * 10],
                        scalar1=lns[:, g:g + 1], scalar2=None,
                        op0=AluOpType.subtract)
                orow = sl * _SLICE
                nc.sync.dma_start(
                    out=out_d[orow:orow + _SLICE, :].rearrange(
                        "(g p) k -> p g k", p=128),
                    in_=ot.rearrange("p (g k) -> p g k", k=10))
    nc.compile()
    return nc


_PROGRAM_CACHE = {}


def kernel(x, w1, b1, w2, b2, fw1, fb1, fw2, fb2):
    global LAST_RESULTS
    tile_wins, consts, tiles = _host_prep(x, w1, b1, w2, b2, fw1, fb1, fw2, fb2)

    if "nc" not in _PROGRAM_CACHE:
        _PROGRAM_CACHE["nc"] = _build_program(tiles, consts["cls_idx"])
    nc = _PROGRAM_CACHE["nc"]

    f8 = ml_dtypes.float8_e4m3fn
    n_quads = len(tiles) // 4
    shared = {k: consts[k] for k in ("w1quad", "wcpack", "cb32", "fw2t")}
    in_maps = []
    for c in range(_NCORES):
        m = dict(shared)
        blob = np.zeros((128, _NSL * n_quads * 1024), f8)
        for sl in range(_NSL):
            for qd in range(n_quads):
                col0 = (sl * n_quads + qd) * 1024
                for g in range(4):
                    t = tiles[4 * qd + g]
                    K2 = _CLS_K2[t["cls"]]
                    w = tile_wins[4 * qd + g]     # [2, K2, B]
                    c0 = c * _PC + sl * _SLICE
                    for j in range(2):
                        blob[32 * g:32 * g + K2,
                             col0 + j * _SLICE:col0 + (j + 1) * _SLICE] = \
                            w[j, :, c0:c0 + _SLICE]
        m["xwin"] = blob
        in_maps.append(m)

    from concourse.bass_utils import run_bass_kernel_spmd
    trace = bool(int(os.environ.get("BASS_KERNEL_TRACE", "0")))
    res = run_bass_kernel_spmd(nc, in_maps, core_ids=list(range(_NCORES)),
                               trace=trace)
    LAST_RESULTS = res
    return np.concatenate([r["out"] for r in res.results], axis=0)
